# revision 26
# baseline (speedup 1.0000x reference)
"""Trainium2 Bass kernel for nn_ChebKernelMixture.

Computes gram(xs) = psi(xs) @ psi(xs).T where psi is a Chebyshev feature
map: psi(x) = concat_n sqrt(w_n) * phi_n(x), phi_0 = [1],
phi_n = [T_n(x), sqrt(1-x^2) U_{n-1}(x)], w = softmax(logits).

Shapes: xs (16384,), logits (33,) -> out (16384, 16384) f32.

Strategy (8 NeuronCores, SPMD, no collectives):
  - every core receives the full xs (as xs_all) plus its own 2048-row
    slice (as xs_rows); the program is identical on all cores.
  - on-chip: build psi^T (65 x 16384) once per core (Chebyshev recurrence
    on VectorE, feature-major transpose via TensorE, softmax weights
    folded into the PSUM->SBUF copy), plus psi^T of its own rows
    (65 x 2048).
  - each core computes its (2048 x 16384) block of the Gram matrix with
    TensorE matmuls (K=65, fp32r single-pass) and DMAs it out.
  - host concatenates the 8 row blocks.
"""

import sys

if "/opt/trn_rl_repo" not in sys.path:
    sys.path.insert(0, "/opt/trn_rl_repo")

import numpy as np

N_PTS = 16384
MAX_N = 32
N_FEAT = 2 * MAX_N + 1  # 65
N_CORES = 8
ROWS_PER_CORE = N_PTS // N_CORES  # 2048
N_BLOCKS = N_PTS // 128  # 128 column point-blocks
N_ROW_BLOCKS = ROWS_PER_CORE // 128  # 16 row point-blocks

# matmul operand dtype: "f32r" (full-rate fp32, hw rounding), "f32" (exact
# fp32, 4 cycles/row) — switch if f32r numerics miss tolerance.
MM_DTYPE = "f32r"

_CACHE = {}


def _build_nc():
    import concourse.bacc as bacc
    import concourse.tile as tile
    from concourse import mybir
    from concourse.masks import make_identity
    from contextlib import ExitStack

    f32 = mybir.dt.float32
    mm_dt = mybir.dt.float32r if MM_DTYPE == "f32r" else mybir.dt.float32
    Act = mybir.ActivationFunctionType
    Alu = mybir.AluOpType

    nc = bacc.Bacc("TRN2", target_bir_lowering=False, debug=False,
                   num_devices=N_CORES)

    xs_all = nc.dram_tensor("xs_all", [128, 128], f32,
                            kind="ExternalInput").ap()
    xs_rows = nc.dram_tensor("xs_rows", [N_ROW_BLOCKS, 128], f32,
                             kind="ExternalInput").ap()
    logits = nc.dram_tensor("logits", [1, MAX_N + 1], f32,
                            kind="ExternalInput").ap()
    g = nc.dram_tensor("g", [ROWS_PER_CORE, N_PTS], f32,
                       kind="ExternalOutput").ap()

    with tile.TileContext(nc) as tc, ExitStack() as ctx:
        consts = ctx.enter_context(tc.tile_pool(name="consts", bufs=1))
        smalls = ctx.enter_context(tc.tile_pool(name="smalls", bufs=1))
        tmpp = ctx.enter_context(tc.tile_pool(name="tmpp", bufs=2))
        phip = ctx.enter_context(tc.tile_pool(name="phip", bufs=1))
        psip = ctx.enter_context(tc.tile_pool(name="psip", bufs=1))
        outp = ctx.enter_context(tc.tile_pool(name="outp", bufs=4))
        pre_ps = ctx.enter_context(
            tc.tile_pool(name="pre_ps", bufs=2, space="PSUM"))
        mm_ps = ctx.enter_context(
            tc.tile_pool(name="mm_ps", bufs=3, space="PSUM"))

        # ---- input DMAs -------------------------------------------------
        X = smalls.tile([128, 128], f32, tag="X")
        nc.sync.dma_start(X[:], xs_all[:])
        Xr = smalls.tile([N_ROW_BLOCKS, 128], f32, tag="Xr")
        nc.sync.dma_start(Xr[:], xs_rows[:])
        Lg = smalls.tile([1, MAX_N + 1], f32, tag="Lg")
        nc.sync.dma_start(Lg[:], logits[:])

        # ---- constants --------------------------------------------------
        identity = consts.tile([128, 128], f32, tag="identity")
        make_identity(nc, identity[:])
        # dup[j, k] = 1 iff k == 2j or k == 2j-1 (degree-duplication map)
        dup = consts.tile([MAX_N + 1, N_FEAT], f32, tag="dup")
        nc.gpsimd.memset(dup[:], 0.0)
        nc.gpsimd.affine_select(
            out=dup[:], in_=dup[:], compare_op=Alu.not_equal, fill=1.0,
            base=0, pattern=[[-1, N_FEAT]], channel_multiplier=2)
        nc.gpsimd.affine_select(
            out=dup[:], in_=dup[:], compare_op=Alu.not_equal, fill=1.0,
            base=-1, pattern=[[-1, N_FEAT]], channel_multiplier=2)

        # ---- transpose x into point-block-major layout ------------------
        # XtF[:, b]: b in [0, 16) holds the core's own row point-blocks,
        # b in [16, 144) holds column point-block b-16 of the full xs.
        NB = N_BLOCKS + N_ROW_BLOCKS  # 144
        XtF = smalls.tile([128, NB], f32, tag="XtF")
        xtr_ps = pre_ps.tile([128, N_ROW_BLOCKS], f32, tag="pre")
        nc.tensor.transpose(xtr_ps[:], Xr[:],
                            identity[0:N_ROW_BLOCKS, 0:N_ROW_BLOCKS])
        nc.any.tensor_copy(XtF[:, 0:N_ROW_BLOCKS], xtr_ps[:])
        xt_ps = pre_ps.tile([128, 128], f32, tag="pre")
        nc.tensor.transpose(xt_ps[:], X[:], identity[:])
        nc.any.tensor_copy(XtF[:, N_ROW_BLOCKS:NB], xt_ps[:])

        # ---- softmax(logits) -> sqrt weights, expanded per feature -----
        SW65 = smalls.tile([N_FEAT, 1], f32, tag="SW65")

        def softmax_weights():
            E = smalls.tile([1, MAX_N + 1], f32, tag="E")
            nc.scalar.activation(E[:], Lg[:], Act.Exp)
            S = smalls.tile([1, 1], f32, tag="S")
            nc.vector.tensor_reduce(S[:], E[:], axis=mybir.AxisListType.X,
                                    op=Alu.add)
            R = smalls.tile([1, 1], f32, tag="R")
            nc.vector.reciprocal(R[:], S[:])
            W = smalls.tile([1, MAX_N + 1], f32, tag="W")
            nc.vector.tensor_scalar_mul(W[:], E[:], R[:])
            SW = smalls.tile([1, MAX_N + 1], f32, tag="SW")
            nc.scalar.activation(SW[:], W[:], Act.Sqrt)
            # (1, 33) -> (33, 1) via PE transpose, then expand to (65, 1)
            swc_ps = pre_ps.tile([MAX_N + 1, 1], f32, tag="pre")
            nc.tensor.transpose(swc_ps[:], SW[:], identity[0:1, 0:1])
            SWc = smalls.tile([MAX_N + 1, 1], f32, tag="SWc")
            nc.any.tensor_copy(SWc[:], swc_ps[:])
            sw65_ps = pre_ps.tile([N_FEAT, 1], f32, tag="pre")
            nc.tensor.matmul(sw65_ps[:], dup[:], SWc[:], start=True,
                             stop=True)
            nc.any.tensor_copy(SW65[:], sw65_ps[:])

        # ---- Chebyshev recurrence (features in PHI, point-block layout) -
        # feature order: 0 -> 1;  2n-1 -> T_n;  2n -> s*U_{n-1}
        # Processed in free-dim chunks so transposes/GEMM on early blocks
        # overlap with recurrence on later blocks.
        x2 = smalls.tile([128, NB], f32, tag="x2")
        x2d2 = smalls.tile([128, 2, NB], f32, tag="x2d2")
        PHI = phip.tile([128, N_FEAT, NB], f32, tag="PHI")
        # single psi^T buffer: block b of XtF lands at cols [b*128,
        # (b+1)*128) — rows (b < 16) then full-xs column blocks. Keeping
        # them adjacent lets one eviction op cover 4 transposes.
        psiA = psip.tile([N_FEAT, NB * 128], mm_dt, tag="psiA")

        def rec_chunk(c0, c1):
            nc.vector.tensor_mul(x2[:, c0:c1], XtF[:, c0:c1], XtF[:, c0:c1])
            nc.vector.tensor_scalar_mul(x2d2[:, 0, c0:c1], XtF[:, c0:c1],
                                        2.0)
            nc.vector.tensor_scalar_mul(x2d2[:, 1, c0:c1], XtF[:, c0:c1],
                                        2.0)
            nc.vector.memset(PHI[:, 0, c0:c1], 1.0)
            nc.vector.tensor_copy(PHI[:, 1, c0:c1], XtF[:, c0:c1])  # T_1
            # s = sqrt(1 - x^2)  (|x| <= 1 so the argument >= 0 in fp32)
            nc.scalar.activation(PHI[:, 2, c0:c1], x2[:, c0:c1], Act.Sqrt,
                                 bias=1.0, scale=-1.0)       # s*U_0 = s
            nc.vector.tensor_scalar(PHI[:, 3, c0:c1], x2[:, c0:c1], 2.0,
                                    -1.0, op0=Alu.mult, op1=Alu.add)  # T_2
            nc.vector.tensor_mul(PHI[:, 4, c0:c1], x2d2[:, 0, c0:c1],
                                 PHI[:, 2, c0:c1])           # s*U_1 = 2x*s
            # pairwise: (T_n, s*U_{n-1}) = 2x*(T_{n-1}, s*U_{n-2})
            #                              - (T_{n-2}, s*U_{n-3})
            for n in range(3, MAX_N + 1):
                tmp = tmpp.tile([128, 2, NB], f32, tag="tmp")
                nc.vector.tensor_mul(tmp[:, :, c0:c1],
                                     PHI[:, 2 * n - 3:2 * n - 1, c0:c1],
                                     x2d2[:, :, c0:c1])
                nc.vector.tensor_sub(PHI[:, 2 * n - 1:2 * n + 1, c0:c1],
                                     tmp[:, :, c0:c1],
                                     PHI[:, 2 * n - 5:2 * n - 3, c0:c1])

        def transposes(b0, b1):
            # psi^T blocks carry the sqrt(w) row scaling, folded into the
            # PSUM->SBUF eviction (ScalarE, keeps VectorE on the
            # recurrence). Up to 4 transposes share one PSUM tile and one
            # eviction op.
            b = b0
            while b < b1:
                g_ = min(4, b1 - b)
                tps = pre_ps.tile([N_FEAT, g_ * 128], f32, tag="pre")
                for i in range(g_):
                    nc.tensor.transpose(tps[:, i * 128:(i + 1) * 128],
                                        PHI[:, :, b + i], identity[:])
                nc.scalar.mul(psiA[:, b * 128:(b + g_) * 128], tps[:],
                              SW65[:])
                b += g_

        def strips_cols(col0, col1, after_m=None, pre_m=None):
            # output strips covering cols [col0, col1) for all 16 row
            # tiles; matmuls paired into a 2-bank PSUM tile so each
            # eviction copy moves 1024 columns in one op. after_m(m)
            # lets transposes for later blocks interleave into the PE
            # stream so DMA never starves while PE transposes.
            width = col1 - col0
            for m in range(N_ROW_BLOCKS):
                if pre_m is not None:
                    pre_m(m)
                lhsT = psiA[:, m * 128:(m + 1) * 128]
                strip = outp.tile([128, width], f32, tag="strip")
                for j in range(width // 1024):
                    c = ROWS_PER_CORE + col0 + j * 1024
                    ps = mm_ps.tile([128, 1024], f32, tag="ps")
                    nc.tensor.matmul(ps[:, 0:512], lhsT,
                                     psiA[:, c:c + 512],
                                     start=True, stop=True)
                    nc.tensor.matmul(ps[:, 512:1024], lhsT,
                                     psiA[:, c + 512:c + 1024],
                                     start=True, stop=True)
                    nc.any.tensor_copy(
                        strip[:, j * 1024:(j + 1) * 1024], ps[:])
                # alternate between the two HWDGE rings (SP and ACT) so
                # per-DMA setup latency pipelines across rings
                dma_eng = nc.sync if m % 2 == 0 else nc.scalar
                dma_eng.dma_start(
                    g[m * 128:(m + 1) * 128, col0:col1], strip[:])
                if after_m is not None:
                    after_m(m)

        # pipelined emission: rows + first col chunks -> first transposes
        # -> first strips, with later recurrence chunks overlapping
        rec_chunk(0, 32)        # row blocks + col blocks 0..15
        softmax_weights()
        transposes(16, 32)      # col blocks 0..15
        rec_chunk(32, 48)       # col blocks 16..31
        rec_chunk(48, 112)      # col blocks 32..95
        # row-block transposes ride just ahead of the strip that needs
        # them; later col-block transposes are spread 1-2 per strip so
        # strip production (PE) stays ahead of the output DMA.
        strips_cols(0, 2048,
                    pre_m=lambda m: transposes(m, m + 4)
                    if m % 4 == 0 else None,
                    after_m=lambda m: transposes(29 + m, 33 + m)
                    if m % 4 == 3 else None)
        strips_cols(2048, 4096,
                    after_m=lambda m: transposes(46 + 2 * m, 50 + 2 * m)
                    if m % 2 == 1 else None)
        rec_chunk(112, 144)     # col blocks 96..127
        strips_cols(4096, 8192,
                    after_m=lambda m: transposes(78 + 2 * m, 82 + 2 * m)
                    if m % 2 == 1 else None)
        strips_cols(8192, 12288,
                    after_m=lambda m: transposes(110 + 2 * m, 114 + 2 * m)
                    if m % 2 == 1 else None)
        strips_cols(12288, 16384)

    nc.compile()
    return nc


def _get_nc():
    if "nc" not in _CACHE:
        _CACHE["nc"] = _build_nc()
    return _CACHE["nc"]


def _make_in_maps(xs, logits):
    xs = np.ascontiguousarray(np.asarray(xs, dtype=np.float32).reshape(N_PTS))
    lg = np.ascontiguousarray(
        np.asarray(logits, dtype=np.float32).reshape(1, MAX_N + 1))
    xa = xs.reshape(128, 128)
    in_maps = []
    for c in range(N_CORES):
        in_maps.append({
            "xs_all": xa,
            "xs_rows": xs[c * ROWS_PER_CORE:(c + 1) * ROWS_PER_CORE]
            .reshape(N_ROW_BLOCKS, 128).copy(),
            "logits": lg,
        })
    return in_maps


def run(xs, logits, trace=False, tmpdir=None):
    """Run the SPMD kernel; returns (full output, BassKernelResults)."""
    from concourse.bass_utils import run_bass_kernel_spmd

    nc = _get_nc()
    in_maps = _make_in_maps(xs, logits)
    res = run_bass_kernel_spmd(nc, in_maps, list(range(N_CORES)),
                               trace=trace, tmpdir=tmpdir)
    out = np.concatenate(
        [res.results[c]["g"] for c in range(N_CORES)], axis=0)
    return out, res


def kernel(xs, logits):
    out, _ = run(xs, logits, trace=False)
    return out


# revision 27
# speedup vs baseline: 1.0369x; 1.0369x over previous
"""Trainium2 Bass kernel for nn_ChebKernelMixture.

Computes gram(xs) = psi(xs) @ psi(xs).T where psi is a Chebyshev feature
map: psi(x) = concat_n sqrt(w_n) * phi_n(x), phi_0 = [1],
phi_n = [T_n(x), sqrt(1-x^2) U_{n-1}(x)], w = softmax(logits).

Shapes: xs (16384,), logits (33,) -> out (16384, 16384) f32.

Strategy (8 NeuronCores, SPMD, no collectives):
  - every core receives the full xs (as xs_all) plus its own 2048-row
    slice (as xs_rows); the program is identical on all cores.
  - on-chip: build psi^T (65 x 16384) once per core (Chebyshev recurrence
    on VectorE, feature-major transpose via TensorE, softmax weights
    folded into the PSUM->SBUF copy), plus psi^T of its own rows
    (65 x 2048).
  - each core computes its (2048 x 16384) block of the Gram matrix with
    TensorE matmuls (K=65, fp32r single-pass) and DMAs it out.
  - host concatenates the 8 row blocks.
"""

import sys

if "/opt/trn_rl_repo" not in sys.path:
    sys.path.insert(0, "/opt/trn_rl_repo")

import numpy as np

N_PTS = 16384
MAX_N = 32
N_FEAT = 2 * MAX_N + 1  # 65
N_CORES = 8
ROWS_PER_CORE = N_PTS // N_CORES  # 2048
N_BLOCKS = N_PTS // 128  # 128 column point-blocks
N_ROW_BLOCKS = ROWS_PER_CORE // 128  # 16 row point-blocks

# matmul operand dtype: "f32r" (full-rate fp32, hw rounding), "f32" (exact
# fp32, 4 cycles/row) — switch if f32r numerics miss tolerance.
MM_DTYPE = "f32r"

_CACHE = {}


def _build_nc():
    import concourse.bacc as bacc
    import concourse.tile as tile
    from concourse import mybir
    from concourse.masks import make_identity
    from contextlib import ExitStack

    f32 = mybir.dt.float32
    mm_dt = mybir.dt.float32r if MM_DTYPE == "f32r" else mybir.dt.float32
    Act = mybir.ActivationFunctionType
    Alu = mybir.AluOpType

    nc = bacc.Bacc("TRN2", target_bir_lowering=False, debug=False,
                   num_devices=N_CORES)

    xs_all = nc.dram_tensor("xs_all", [128, 128], f32,
                            kind="ExternalInput").ap()
    xs_rows = nc.dram_tensor("xs_rows", [N_ROW_BLOCKS, 128], f32,
                             kind="ExternalInput").ap()
    logits = nc.dram_tensor("logits", [1, MAX_N + 1], f32,
                            kind="ExternalInput").ap()
    g = nc.dram_tensor("g", [ROWS_PER_CORE, N_PTS], f32,
                       kind="ExternalOutput").ap()

    with tile.TileContext(nc) as tc, ExitStack() as ctx:
        consts = ctx.enter_context(tc.tile_pool(name="consts", bufs=1))
        smalls = ctx.enter_context(tc.tile_pool(name="smalls", bufs=1))
        tmpp = ctx.enter_context(tc.tile_pool(name="tmpp", bufs=2))
        phip = ctx.enter_context(tc.tile_pool(name="phip", bufs=1))
        psip = ctx.enter_context(tc.tile_pool(name="psip", bufs=1))
        outp = ctx.enter_context(tc.tile_pool(name="outp", bufs=3))
        pre_ps = ctx.enter_context(
            tc.tile_pool(name="pre_ps", bufs=2, space="PSUM"))
        mm_ps = ctx.enter_context(
            tc.tile_pool(name="mm_ps", bufs=3, space="PSUM"))

        # ---- input DMAs -------------------------------------------------
        X = smalls.tile([128, 128], f32, tag="X")
        nc.sync.dma_start(X[:], xs_all[:])
        Xr = smalls.tile([N_ROW_BLOCKS, 128], f32, tag="Xr")
        nc.sync.dma_start(Xr[:], xs_rows[:])
        Lg = smalls.tile([1, MAX_N + 1], f32, tag="Lg")
        nc.sync.dma_start(Lg[:], logits[:])

        # ---- constants --------------------------------------------------
        identity = consts.tile([128, 128], f32, tag="identity")
        make_identity(nc, identity[:])
        # dup[j, k] = 1 iff k == 2j or k == 2j-1 (degree-duplication map)
        dup = consts.tile([MAX_N + 1, N_FEAT], f32, tag="dup")
        nc.gpsimd.memset(dup[:], 0.0)
        nc.gpsimd.affine_select(
            out=dup[:], in_=dup[:], compare_op=Alu.not_equal, fill=1.0,
            base=0, pattern=[[-1, N_FEAT]], channel_multiplier=2)
        nc.gpsimd.affine_select(
            out=dup[:], in_=dup[:], compare_op=Alu.not_equal, fill=1.0,
            base=-1, pattern=[[-1, N_FEAT]], channel_multiplier=2)

        # ---- transpose x into point-block-major layout ------------------
        # XtF[:, b]: b in [0, 16) holds the core's own row point-blocks,
        # b in [16, 144) holds column point-block b-16 of the full xs.
        NB = N_BLOCKS + N_ROW_BLOCKS  # 144
        XtF = smalls.tile([128, NB], f32, tag="XtF")
        xtr_ps = pre_ps.tile([128, N_ROW_BLOCKS], f32, tag="pre")
        nc.tensor.transpose(xtr_ps[:], Xr[:],
                            identity[0:N_ROW_BLOCKS, 0:N_ROW_BLOCKS])
        nc.any.tensor_copy(XtF[:, 0:N_ROW_BLOCKS], xtr_ps[:])
        xt_ps = pre_ps.tile([128, 128], f32, tag="pre")
        nc.tensor.transpose(xt_ps[:], X[:], identity[:])
        nc.any.tensor_copy(XtF[:, N_ROW_BLOCKS:NB], xt_ps[:])

        # ---- softmax(logits) -> sqrt weights, expanded per feature -----
        SW65 = smalls.tile([N_FEAT, 1], f32, tag="SW65")

        def softmax_weights():
            E = smalls.tile([1, MAX_N + 1], f32, tag="E")
            nc.scalar.activation(E[:], Lg[:], Act.Exp)
            S = smalls.tile([1, 1], f32, tag="S")
            nc.vector.tensor_reduce(S[:], E[:], axis=mybir.AxisListType.X,
                                    op=Alu.add)
            R = smalls.tile([1, 1], f32, tag="R")
            nc.vector.reciprocal(R[:], S[:])
            W = smalls.tile([1, MAX_N + 1], f32, tag="W")
            nc.vector.tensor_scalar_mul(W[:], E[:], R[:])
            SW = smalls.tile([1, MAX_N + 1], f32, tag="SW")
            nc.scalar.activation(SW[:], W[:], Act.Sqrt)
            # (1, 33) -> (33, 1) via PE transpose, then expand to (65, 1)
            swc_ps = pre_ps.tile([MAX_N + 1, 1], f32, tag="pre")
            nc.tensor.transpose(swc_ps[:], SW[:], identity[0:1, 0:1])
            SWc = smalls.tile([MAX_N + 1, 1], f32, tag="SWc")
            nc.any.tensor_copy(SWc[:], swc_ps[:])
            sw65_ps = pre_ps.tile([N_FEAT, 1], f32, tag="pre")
            nc.tensor.matmul(sw65_ps[:], dup[:], SWc[:], start=True,
                             stop=True)
            nc.any.tensor_copy(SW65[:], sw65_ps[:])

        # ---- Chebyshev recurrence (features in PHI, point-block layout) -
        # feature order: 0 -> 1;  2n-1 -> T_n;  2n -> s*U_{n-1}
        # Processed in free-dim chunks so transposes/GEMM on early blocks
        # overlap with recurrence on later blocks.
        x2 = smalls.tile([128, NB], f32, tag="x2")
        x2d2 = smalls.tile([128, 2, NB], f32, tag="x2d2")
        PHI = phip.tile([128, N_FEAT, NB], f32, tag="PHI")
        # single psi^T buffer: block b of XtF lands at cols [b*128,
        # (b+1)*128) — rows (b < 16) then full-xs column blocks. Keeping
        # them adjacent lets one eviction op cover 4 transposes.
        psiA = psip.tile([N_FEAT, NB * 128], mm_dt, tag="psiA")

        def rec_chunk(c0, c1):
            nc.vector.tensor_mul(x2[:, c0:c1], XtF[:, c0:c1], XtF[:, c0:c1])
            nc.vector.tensor_scalar_mul(x2d2[:, 0, c0:c1], XtF[:, c0:c1],
                                        2.0)
            nc.vector.tensor_scalar_mul(x2d2[:, 1, c0:c1], XtF[:, c0:c1],
                                        2.0)
            nc.vector.memset(PHI[:, 0, c0:c1], 1.0)
            nc.vector.tensor_copy(PHI[:, 1, c0:c1], XtF[:, c0:c1])  # T_1
            # s = sqrt(1 - x^2)  (|x| <= 1 so the argument >= 0 in fp32)
            nc.scalar.activation(PHI[:, 2, c0:c1], x2[:, c0:c1], Act.Sqrt,
                                 bias=1.0, scale=-1.0)       # s*U_0 = s
            nc.vector.tensor_scalar(PHI[:, 3, c0:c1], x2[:, c0:c1], 2.0,
                                    -1.0, op0=Alu.mult, op1=Alu.add)  # T_2
            nc.vector.tensor_mul(PHI[:, 4, c0:c1], x2d2[:, 0, c0:c1],
                                 PHI[:, 2, c0:c1])           # s*U_1 = 2x*s
            # pairwise: (T_n, s*U_{n-1}) = 2x*(T_{n-1}, s*U_{n-2})
            #                              - (T_{n-2}, s*U_{n-3})
            for n in range(3, MAX_N + 1):
                tmp = tmpp.tile([128, 2, NB], f32, tag="tmp")
                nc.vector.tensor_mul(tmp[:, :, c0:c1],
                                     PHI[:, 2 * n - 3:2 * n - 1, c0:c1],
                                     x2d2[:, :, c0:c1])
                nc.vector.tensor_sub(PHI[:, 2 * n - 1:2 * n + 1, c0:c1],
                                     tmp[:, :, c0:c1],
                                     PHI[:, 2 * n - 5:2 * n - 3, c0:c1])

        def transposes(b0, b1):
            # psi^T blocks carry the sqrt(w) row scaling, folded into the
            # PSUM->SBUF eviction (ScalarE, keeps VectorE on the
            # recurrence). Up to 4 transposes share one PSUM tile and one
            # eviction op.
            b = b0
            while b < b1:
                g_ = min(4, b1 - b)
                tps = pre_ps.tile([N_FEAT, g_ * 128], f32, tag="pre")
                for i in range(g_):
                    nc.tensor.transpose(tps[:, i * 128:(i + 1) * 128],
                                        PHI[:, :, b + i], identity[:])
                nc.scalar.mul(psiA[:, b * 128:(b + g_) * 128], tps[:],
                              SW65[:])
                b += g_

        def strips_cols(col0, col1, after_m=None, pre_m=None):
            # output strips covering cols [col0, col1) for all 16 row
            # tiles; matmuls paired into a 2-bank PSUM tile so each
            # eviction copy moves 1024 columns in one op. after_m(m)
            # lets transposes for later blocks interleave into the PE
            # stream so DMA never starves while PE transposes.
            width = col1 - col0
            for m in range(N_ROW_BLOCKS):
                if pre_m is not None:
                    pre_m(m)
                lhsT = psiA[:, m * 128:(m + 1) * 128]
                strip = outp.tile([128, width], f32, tag="strip")
                for j in range(width // 1024):
                    c = ROWS_PER_CORE + col0 + j * 1024
                    ps = mm_ps.tile([128, 1024], f32, tag="ps")
                    nc.tensor.matmul(ps[:, 0:512], lhsT,
                                     psiA[:, c:c + 512],
                                     start=True, stop=True)
                    nc.tensor.matmul(ps[:, 512:1024], lhsT,
                                     psiA[:, c + 512:c + 1024],
                                     start=True, stop=True)
                    nc.any.tensor_copy(
                        strip[:, j * 1024:(j + 1) * 1024], ps[:])
                # alternate between the two HWDGE rings (SP and ACT) so
                # per-DMA setup latency pipelines across rings
                dma_eng = nc.sync if m % 2 == 0 else nc.scalar
                dma_eng.dma_start(
                    g[m * 128:(m + 1) * 128, col0:col1], strip[:])
                if after_m is not None:
                    after_m(m)

        # pipelined emission: rows + first col chunks -> first transposes
        # -> first strips, with later recurrence chunks overlapping
        rec_chunk(0, 32)        # row blocks + col blocks 0..15
        softmax_weights()
        transposes(16, 32)      # col blocks 0..15
        rec_chunk(32, 48)       # col blocks 16..31
        rec_chunk(48, 112)      # col blocks 32..95
        # row-block transposes ride just ahead of the strip that needs
        # them; later col-block transposes are spread 1-2 per strip so
        # strip production (PE) stays ahead of the output DMA.
        strips_cols(0, 2048,
                    pre_m=lambda m: transposes(m, m + 4)
                    if m % 4 == 0 else None,
                    after_m=lambda m: transposes(29 + m, 33 + m)
                    if m % 4 == 3 else None)
        strips_cols(2048, 4096,
                    after_m=lambda m: transposes(46 + 2 * m, 50 + 2 * m)
                    if m % 2 == 1 else None)
        rec_chunk(112, 144)     # col blocks 96..127
        strips_cols(4096, 8192,
                    after_m=lambda m: transposes(78 + 2 * m, 82 + 2 * m)
                    if m % 2 == 1 else None)
        strips_cols(8192, 12288,
                    after_m=lambda m: transposes(110 + 2 * m, 114 + 2 * m)
                    if m % 2 == 1 else None)
        strips_cols(12288, 16384)

    nc.compile()
    return nc


def _get_nc():
    if "nc" not in _CACHE:
        _CACHE["nc"] = _build_nc()
    return _CACHE["nc"]


def _make_in_maps(xs, logits):
    xs = np.ascontiguousarray(np.asarray(xs, dtype=np.float32).reshape(N_PTS))
    lg = np.ascontiguousarray(
        np.asarray(logits, dtype=np.float32).reshape(1, MAX_N + 1))
    xa = xs.reshape(128, 128)
    in_maps = []
    for c in range(N_CORES):
        in_maps.append({
            "xs_all": xa,
            "xs_rows": xs[c * ROWS_PER_CORE:(c + 1) * ROWS_PER_CORE]
            .reshape(N_ROW_BLOCKS, 128).copy(),
            "logits": lg,
        })
    return in_maps


def run(xs, logits, trace=False, tmpdir=None):
    """Run the SPMD kernel; returns (full output, BassKernelResults)."""
    from concourse.bass_utils import run_bass_kernel_spmd

    nc = _get_nc()
    in_maps = _make_in_maps(xs, logits)
    res = run_bass_kernel_spmd(nc, in_maps, list(range(N_CORES)),
                               trace=trace, tmpdir=tmpdir)
    out = np.concatenate(
        [res.results[c]["g"] for c in range(N_CORES)], axis=0)
    return out, res


def kernel(xs, logits):
    out, _ = run(xs, logits, trace=False)
    return out
